# revision 1
# baseline (speedup 1.0000x reference)
"""GCN-style 8-step SpMM power iteration on 8 Trainium2 NeuronCores.

Math (reference):
    deg = segment_sum(1, col); dis = rsqrt(max(deg,1)) where deg>0 else 0
    norm_e = dis[row_e] * dis[col_e];  row' = row - row.min()
    xX = x @ W_linX + b_linX
    hX_{t+1}[v] = sum_{e: row'_e = v} norm_e * hX_t[col_e] + xX[v]   (8 times)
    out = relu(pp0*xX + pp1*hX_8) @ W_pred + b_pred

Key algebraic trick: norm factorizes per-edge into src/dst node factors, so we
keep the node table pre-scaled: T = dis ⊙ hX. Then one step is
    S[v]   = sum_{e->v} T[col_e]            (pure gather + segment-sum, no
                                             per-edge arithmetic at all)
    hX'[v] = dis_sh[v]*S[v] + xX[v]         (dis_sh = dis shifted by row.min())
    T'[v]  = dis[v]*hX'[v] = (dis*dis_sh)[v]*S[v] + dis[v]*xX[v]

Distribution: nodes dst-sharded over 8 cores (node v -> core v // (N/8)).
Each core owns a contiguous slice of a relabeled "slot" table; per-iteration
AllGather rebuilds the full table on every core. Gather of source rows uses
dma_gather (int16 indices -> table split in two halves; edges partitioned by
source half). Segment-sum runs on the TensorEngine: edges are binned into
sub-blocks of <=32 destination nodes with a fixed budget of 2 chunks (128
edges each) per source-half; each chunk's 0/1 selection matrix S (fp16,
host-built) is the stationary matmul operand, the gathered fp16 messages the
moving one, accumulating fp32 in PSUM.
"""

import numpy as np

# problem shape (hardcoded per the task contract)
N = 50000
E = 800000
IN_C = 128
HID = 128
OUT_C = 40
POWER1 = 8

NCORES = 8
SUB_NODES = 32          # destination slots per sub-block (= matmul M)
CHUNK = 128             # edges per chunk (= matmul K)
A_CHUNKS = 2            # chunks per sub-block from source half A
B_CHUNKS = 2
BATCH_GROUPS = 6        # psum groups (of 4 sub-blocks) per gather batch
GROUP_SUBS = 4          # sub-blocks per psum group ([128,128] psum tile)


# ----------------------------------------------------------------------------
# Host-side preprocessing
# ----------------------------------------------------------------------------

def _pack_core(degA, degB, capA, capB, sub_nodes):
    """2D best-fit-decreasing bin packing of nodes into sub-blocks."""
    order = np.argsort(-np.maximum(degA, degB), kind="stable")
    bins = []        # (node_list, sumA, sumB)
    for v in order:
        a, b = int(degA[v]), int(degB[v])
        best, best_slack = -1, None
        for i, (nodes, sa, sb) in enumerate(bins):
            if len(nodes) < sub_nodes and sa + a <= capA and sb + b <= capB:
                slack = (capA - sa - a) + (capB - sb - b)
                if best_slack is None or slack < best_slack:
                    best, best_slack = i, slack
        if best < 0:
            bins.append(([v], a, b))
        else:
            nodes, sa, sb = bins[best]
            nodes.append(v)
            bins[best] = (nodes, sa + a, sb + b)
    return [b[0] for b in bins]


def _preprocess(inputs, n=N, ncores=NCORES):
    x = np.asarray(inputs["x"], dtype=np.float32)
    edge_index = np.asarray(inputs["edge_index"])
    W_linX = np.asarray(inputs["W_linX"], dtype=np.float32)
    b_linX = np.asarray(inputs["b_linX"], dtype=np.float32)
    policy = np.asarray(inputs["policy"], dtype=np.float64)
    W_pred = np.asarray(inputs["W_pred"], dtype=np.float32)
    b_pred = np.asarray(inputs["b_pred"], dtype=np.float32)

    npc = n // ncores
    row = edge_index[0].astype(np.int64)
    col = edge_index[1].astype(np.int64)
    deg = np.bincount(col, minlength=n).astype(np.float64)
    dis = np.where(deg > 0, 1.0 / np.sqrt(np.maximum(deg, 1.0)), 0.0)
    shift = int(row.min())
    dst = row - shift                      # aggregation destination
    # per-dst factor is dis at the *unshifted* row id
    dis_sh = np.zeros(n, dtype=np.float64)
    hi = n - shift
    dis_sh[:hi] = dis[shift:]

    e = np.exp(policy[:2] - policy[:2].max())
    pp = e / e.sum()
    pp0, pp1 = float(pp[0]), float(pp[1])
    b_comb = pp0 + pp1                      # == 1.0, but don't rely on it

    half_nodes = (ncores // 2) * npc        # src half A = nodes < half_nodes
    capA, capB = A_CHUNKS * CHUNK, B_CHUNKS * CHUNK

    cores = []
    max_bins = 0
    for c in range(ncores):
        m = (dst >= c * npc) & (dst < (c + 1) * npc)
        e_dst = dst[m] - c * npc
        e_src = col[m]
        degA = np.bincount(e_dst[e_src < half_nodes], minlength=npc)
        degB = np.bincount(e_dst[e_src >= half_nodes], minlength=npc)
        bins = _pack_core(degA, degB, capA, capB, SUB_NODES)
        max_bins = max(max_bins, len(bins))
        cores.append((e_dst, e_src, bins))

    lcm = GROUP_SUBS * BATCH_GROUPS         # n_sub must divide into batches
    n_sub = -(-max_bins // lcm) * lcm
    slots = n_sub * SUB_NODES               # table rows per core
    tot = slots * ncores
    half_rows = tot // 2
    assert half_rows <= 32767, f"table half {half_rows} exceeds int16 range"
    n_grp = n_sub // GROUP_SUBS

    # slot assignment + global row map
    slot_of_node = np.full(n, -1, dtype=np.int64)
    for c, (e_dst, e_src, bins) in enumerate(cores):
        for bi, nodes in enumerate(bins):
            for k, v in enumerate(nodes):
                slot_of_node[c * npc + v] = c * slots + bi * SUB_NODES + k
    # an empty slot per core for padding gathers (guaranteed zero row)
    pad_row = np.zeros(ncores, dtype=np.int64)
    for c in range(ncores):
        used = np.zeros(slots, dtype=bool)
        sl = slot_of_node[c * npc:(c + 1) * npc] - c * slots
        used[sl[sl >= 0]] = True
        free = np.flatnonzero(~used)
        assert free.size > 0
        pad_row[c] = c * slots + free[0]

    per_core = []
    nchA, nchB = n_sub * A_CHUNKS, n_sub * B_CHUNKS
    for c, (e_dst, e_src, bins) in enumerate(cores):
        loc = slot_of_node[e_dst + c * npc] - c * slots
        dst_bin = loc // SUB_NODES
        dst_k = loc % SUB_NODES
        isA = e_src < half_nodes
        S = np.zeros((nchA + nchB, CHUNK, SUB_NODES), dtype=np.float16)
        idxA = np.full(nchA * CHUNK, pad_row[0], dtype=np.int64)
        idxB = np.full(nchB * CHUNK, pad_row[ncores // 2] - half_rows,
                       dtype=np.int64)
        for bi in range(n_sub):
            for half in (True, False):
                sel = (dst_bin == bi) & (isA == half)
                srcs = e_src[sel]
                dks = dst_k[sel]
                kk = len(srcs)
                cap = capA if half else capB
                assert kk <= cap, (c, bi, half, kk)
                if half:
                    cbase, sbase, idx, base = bi * A_CHUNKS, 0, idxA, 0
                else:
                    cbase, sbase, idx, base = bi * B_CHUNKS, nchA, idxB, half_rows
                for j in range(kk):
                    ch = cbase + j // CHUNK
                    S[sbase + ch, j % CHUNK, dks[j]] = 1.0
                rows = slot_of_node[srcs] - base
                idx[cbase * CHUNK: cbase * CHUNK + kk] = rows
        assert idxA.min() >= 0 and idxA.max() < half_rows
        assert idxB.min() >= 0 and idxB.max() < tot - half_rows

        def wrap_idx(idx):
            # index i consumed from [i % 16, i // 16]; replicate to 128 parts
            w = idx.reshape(-1, 16).T.astype(np.int16)      # [16, n/16]
            return np.tile(w, (8, 1))                        # [128, n/16]

        # slot-layout host arrays
        x_slot = np.zeros((slots, IN_C), dtype=np.float32)
        dis_slot = np.zeros(slots, dtype=np.float64)
        dsh_slot = np.zeros(slots, dtype=np.float64)
        nodes_c = np.arange(c * npc, (c + 1) * npc)
        sl = slot_of_node[nodes_c] - c * slots
        x_slot[sl] = x[nodes_c]
        dis_slot[sl] = dis[nodes_c]
        dsh_slot[sl] = dis_sh[nodes_c]

        grp = lambda v: v.reshape(n_grp, 128).T.astype(np.float32)
        per_core.append({
            "x_slot": x_slot,
            "S": np.ascontiguousarray(
                S.transpose(1, 0, 2).reshape(CHUNK, -1)),    # [128, TC*32]
            "idxA": wrap_idx(idxA),
            "idxB": wrap_idx(idxB),
            "disg": grp(dis_slot),
            "dis2g": grp(dis_slot * dsh_slot),
            "ag": grp((pp1 / b_comb) * dsh_slot),
            "W_linX": W_linX,
            "bX": np.tile(b_linX[None, :], (128, 1)).astype(np.float32),
            "W_pred": (b_comb * W_pred).astype(np.float32),
            "bP": np.tile(b_pred[None, :], (128, 1)).astype(np.float32),
            "ident": np.eye(128, dtype=np.float32),
        })

    meta = dict(n=n, ncores=ncores, npc=npc, n_sub=n_sub, n_grp=n_grp,
                slots=slots, tot=tot, half_rows=half_rows,
                slot_of_node=slot_of_node)
    return meta, per_core


# ----------------------------------------------------------------------------
# Bass program
# ----------------------------------------------------------------------------

def _build_program(meta, iters=POWER1):
    import concourse.bacc as bacc
    import concourse.mybir as mybir
    from concourse import tile

    f32, f16, i16 = mybir.dt.float32, mybir.dt.float16, mybir.dt.int16
    ADD, MULT = mybir.AluOpType.add, mybir.AluOpType.mult

    ncores = meta["ncores"]
    n_sub, n_grp = meta["n_sub"], meta["n_grp"]
    slots, tot, half_rows = meta["slots"], meta["tot"], meta["half_rows"]
    n_batches = n_grp // BATCH_GROUPS
    nchA = n_sub * A_CHUNKS
    batch_chunks = BATCH_GROUPS * GROUP_SUBS * A_CHUNKS      # 48
    batch_idx = batch_chunks * CHUNK                          # 6144
    TC = n_sub * (A_CHUNKS + B_CHUNKS)

    nc = bacc.Bacc("TRN2", target_bir_lowering=False, debug=False,
                   enable_asserts=False, num_devices=ncores)

    x_slot_h = nc.dram_tensor("x_slot", [slots, IN_C], f32, kind="ExternalInput")
    S_h = nc.dram_tensor("S", [CHUNK, TC * SUB_NODES], f16, kind="ExternalInput")
    idxA_h = nc.dram_tensor("idxA", [128, nchA * CHUNK // 16], i16,
                            kind="ExternalInput")
    idxB_h = nc.dram_tensor("idxB", [128, n_sub * B_CHUNKS * CHUNK // 16], i16,
                            kind="ExternalInput")
    disg_h = nc.dram_tensor("disg", [128, n_grp], f32, kind="ExternalInput")
    dis2g_h = nc.dram_tensor("dis2g", [128, n_grp], f32, kind="ExternalInput")
    ag_h = nc.dram_tensor("ag", [128, n_grp], f32, kind="ExternalInput")
    W_h = nc.dram_tensor("W_linX", [IN_C, HID], f32, kind="ExternalInput")
    bX_h = nc.dram_tensor("bX", [128, HID], f32, kind="ExternalInput")
    Wp_h = nc.dram_tensor("W_pred", [HID, OUT_C], f32, kind="ExternalInput")
    bP_h = nc.dram_tensor("bP", [128, OUT_C], f32, kind="ExternalInput")
    id_h = nc.dram_tensor("ident", [128, 128], f32, kind="ExternalInput")

    tabA = nc.dram_tensor("tabA", [tot, HID], f16, addr_space="Shared")
    tabB = nc.dram_tensor("tabB", [tot, HID], f16, addr_space="Shared")
    shard = nc.dram_tensor("shard", [slots, HID], f16)
    out_h = nc.dram_tensor("out", [slots, OUT_C], f32, kind="ExternalOutput")

    rg = [list(range(ncores))]

    with tile.TileContext(nc, num_cores=ncores) as tc:
        import contextlib
        with contextlib.ExitStack() as ctx:
            cpool = ctx.enter_context(tc.tile_pool(name="const", bufs=1))
            wpool = ctx.enter_context(tc.tile_pool(name="work", bufs=2))
            spool = ctx.enter_context(tc.tile_pool(name="stage", bufs=3))
            ppool = ctx.enter_context(
                tc.tile_pool(name="psum", bufs=4, space="PSUM"))
            tpool = ctx.enter_context(
                tc.tile_pool(name="psum2", bufs=2, space="PSUM"))

            # persistent SBUF
            S_sb = cpool.tile([CHUNK, TC * SUB_NODES], f16)
            nc.sync.dma_start(S_sb[:, :], S_h[:, :])
            idxA_sb = cpool.tile([128, nchA * CHUNK // 16], i16)
            nc.sync.dma_start(idxA_sb[:, :], idxA_h[:, :])
            idxB_sb = cpool.tile([128, n_sub * B_CHUNKS * CHUNK // 16], i16)
            nc.sync.dma_start(idxB_sb[:, :], idxB_h[:, :])
            disg = cpool.tile([128, n_grp], f32)
            nc.sync.dma_start(disg[:, :], disg_h[:, :])
            dis2g = cpool.tile([128, n_grp], f32)
            nc.sync.dma_start(dis2g[:, :], dis2g_h[:, :])
            ag = cpool.tile([128, n_grp], f32)
            nc.sync.dma_start(ag[:, :], ag_h[:, :])
            W_sb = cpool.tile([IN_C, HID], f32)
            nc.sync.dma_start(W_sb[:, :], W_h[:, :])
            bX_sb = cpool.tile([128, HID], f32)
            nc.sync.dma_start(bX_sb[:, :], bX_h[:, :])
            Wp_sb = cpool.tile([HID, OUT_C], f32)
            nc.sync.dma_start(Wp_sb[:, :], Wp_h[:, :])
            bP_sb = cpool.tile([128, OUT_C], f32)
            nc.sync.dma_start(bP_sb[:, :], bP_h[:, :])
            ident = cpool.tile([128, 128], f32)
            nc.sync.dma_start(ident[:, :], id_h[:, :])
            xX_sb = cpool.tile([128, n_grp * HID], f32)    # computed below

            # ---- prologue: xX = x @ W + b; T0 = dis * xX -> shard -> AG
            for g in range(n_grp):
                rows = slice(g * 128, (g + 1) * 128)
                gc = slice(g * HID, (g + 1) * HID)
                x_t = wpool.tile([128, IN_C], f32, tag="xt")
                nc.sync.dma_start(x_t[:, :], x_slot_h[rows, :])
                tp_ps = tpool.tile([128, 128], f32, tag="tp")
                nc.tensor.transpose(tp_ps[:, :], x_t[:, :], ident[:, :])
                xT_sb = wpool.tile([128, 128], f32, tag="xT")
                nc.vector.tensor_copy(xT_sb[:, :], tp_ps[:, :])
                mm_ps = tpool.tile([128, HID], f32, tag="mm2")
                nc.tensor.matmul(mm_ps[:, :], xT_sb[:, :], W_sb[:, :],
                                 start=True, stop=True)
                nc.vector.tensor_tensor(xX_sb[:, gc], mm_ps[:, :],
                                        bX_sb[:, :], op=ADD)
                stage = spool.tile([128, HID], f16, tag="stage")
                nc.vector.tensor_scalar_mul(stage[:, :], xX_sb[:, gc],
                                            disg[:, g:g + 1])
                nc.sync.dma_start(shard[rows, :], stage[:, :])

            nc.gpsimd.collective_compute(
                "AllGather", mybir.AluOpType.bypass, replica_groups=rg,
                ins=[shard.ap().opt()], outs=[tabA.ap().opt()])

            # ---- 8 SpMM iterations
            tables = [tabA, tabB]
            for t in range(iters):
                tin = tables[t % 2]
                last = t == iters - 1
                for b in range(n_batches):
                    mA = wpool.tile([128, batch_idx], f16, tag="mA")
                    nc.gpsimd.dma_gather(
                        mA[:, :].rearrange("p (c e) -> p c e", e=HID),
                        tin[0:half_rows, :],
                        idxA_sb[:, b * (batch_idx // 16):
                                (b + 1) * (batch_idx // 16)],
                        num_idxs=batch_idx, num_idxs_reg=batch_idx,
                        elem_size=HID, single_packet=False)
                    mB = wpool.tile([128, batch_idx], f16, tag="mB")
                    nc.gpsimd.dma_gather(
                        mB[:, :].rearrange("p (c e) -> p c e", e=HID),
                        tin[half_rows:tot, :],
                        idxB_sb[:, b * (batch_idx // 16):
                                (b + 1) * (batch_idx // 16)],
                        num_idxs=batch_idx, num_idxs_reg=batch_idx,
                        elem_size=HID, single_packet=False)
                    for u in range(BATCH_GROUPS):
                        g = b * BATCH_GROUPS + u
                        gc = slice(g * HID, (g + 1) * HID)
                        ps = ppool.tile([128, HID], f32, tag="ps")
                        for j in range(GROUP_SUBS):
                            sb = g * GROUP_SUBS + j
                            prange = slice(32 * j, 32 * j + 32)
                            tpos = (0, 32 * j)
                            for k in range(A_CHUNKS):
                                cA = sb * A_CHUNKS + k
                                q = (u * GROUP_SUBS + j) * A_CHUNKS + k
                                nc.tensor.matmul(
                                    ps[prange, :],
                                    S_sb[:, cA * 32:(cA + 1) * 32],
                                    mA[:, q * HID:(q + 1) * HID],
                                    start=(k == 0), stop=False,
                                    tile_position=tpos)
                            for k in range(B_CHUNKS):
                                cB = nchA + sb * B_CHUNKS + k
                                q = (u * GROUP_SUBS + j) * B_CHUNKS + k
                                nc.tensor.matmul(
                                    ps[prange, :],
                                    S_sb[:, cB * 32:(cB + 1) * 32],
                                    mB[:, q * HID:(q + 1) * HID],
                                    start=False, stop=(k == B_CHUNKS - 1),
                                    tile_position=tpos)
                        rows = slice(g * 128, (g + 1) * 128)
                        if not last:
                            t1 = wpool.tile([128, HID], f32, tag="t1")
                            nc.vector.tensor_scalar_mul(
                                t1[:, :], ps[:, :], dis2g[:, g:g + 1])
                            stage = spool.tile([128, HID], f16, tag="stage")
                            nc.vector.scalar_tensor_tensor(
                                stage[:, :], xX_sb[:, gc], disg[:, g:g + 1],
                                t1[:, :], op0=MULT, op1=ADD)
                            nc.sync.dma_start(shard[rows, :], stage[:, :])
                        else:
                            u_t = wpool.tile([128, HID], f32, tag="t1")
                            nc.vector.scalar_tensor_tensor(
                                u_t[:, :], ps[:, :], ag[:, g:g + 1],
                                xX_sb[:, gc], op0=MULT, op1=ADD)
                            nc.vector.tensor_scalar_max(u_t[:, :], u_t[:, :],
                                                        0.0)
                            tp_ps = tpool.tile([128, 128], f32, tag="tp")
                            nc.tensor.transpose(tp_ps[:, :], u_t[:, :],
                                                ident[:, :])
                            uT_sb = wpool.tile([128, 128], f32, tag="xT")
                            nc.vector.tensor_copy(uT_sb[:, :], tp_ps[:, :])
                            o_ps = tpool.tile([128, OUT_C], f32, tag="mm2")
                            nc.tensor.matmul(o_ps[:, :], uT_sb[:, :],
                                             Wp_sb[:, :], start=True, stop=True)
                            o_sb = spool.tile([128, OUT_C], f32, tag="osb")
                            nc.vector.tensor_tensor(o_sb[:, :], o_ps[:, :],
                                                    bP_sb[:, :], op=ADD)
                            nc.sync.dma_start(out_h[rows, :], o_sb[:, :])
                if not last:
                    nc.gpsimd.collective_compute(
                        "AllGather", mybir.AluOpType.bypass, replica_groups=rg,
                        ins=[shard.ap().opt()],
                        outs=[tables[(t + 1) % 2].ap().opt()])

    nc.compile()
    return nc


# ----------------------------------------------------------------------------
# Runner
# ----------------------------------------------------------------------------

def _run(inputs, n=N, ncores=NCORES, trace=False, use_sim=False, iters=POWER1):
    meta, per_core = _preprocess(inputs, n=n, ncores=ncores)
    nc = _build_program(meta, iters=iters)
    in_maps = [dict(pc) for pc in per_core]

    if use_sim:
        from concourse.bass_interp import MultiCoreSim
        sim = MultiCoreSim(nc, num_cores=ncores)
        for c in range(ncores):
            for k, v in in_maps[c].items():
                sim.cores[c].tensor(k)[:] = v
        sim.simulate(check_with_hw=False)
        results = [{"out": np.array(sim.cores[c].tensor("out"))}
                   for c in range(ncores)]
        bres = None
    else:
        from concourse.bass_utils import run_bass_kernel_spmd
        bres = run_bass_kernel_spmd(nc, in_maps, core_ids=list(range(ncores)),
                                    trace=trace)
        results = bres.results

    # unshard: slots -> nodes
    npc, slots = meta["npc"], meta["slots"]
    son = meta["slot_of_node"]
    out = np.zeros((n, OUT_C), dtype=np.float32)
    for c in range(ncores):
        nodes = np.arange(c * npc, (c + 1) * npc)
        out[nodes] = results[c]["out"][son[nodes] - c * slots]
    return out, bres


def kernel(**inputs) -> np.ndarray:
    # Run twice and compare: guards against rare transient device faults
    # (observed once after an unrecoverable-NRT event on a shared terminal).
    out1, _ = _run(inputs)
    out2, _ = _run(inputs)
    if np.allclose(out1, out2, rtol=0, atol=1e-4):
        return out1
    out3, _ = _run(inputs)
    if np.allclose(out1, out3, rtol=0, atol=1e-4):
        return out1
    return out2 if np.allclose(out2, out3, rtol=0, atol=1e-4) else out3



# revision 3
# speedup vs baseline: 1.5999x; 1.5999x over previous
"""GCN-style 8-step SpMM power iteration on 8 Trainium2 NeuronCores.

Math (reference):
    deg = segment_sum(1, col); dis = rsqrt(max(deg,1)) where deg>0 else 0
    norm_e = dis[row_e] * dis[col_e];  row' = row - row.min()
    xX = x @ W_linX + b_linX
    hX_{t+1}[v] = sum_{e: row'_e = v} norm_e * hX_t[col_e] + xX[v]   (8 times)
    out = relu(pp0*xX + pp1*hX_8) @ W_pred + b_pred

Key algebraic trick: norm factorizes per-edge into src/dst node factors, so we
keep the node table pre-scaled: T = dis ⊙ hX. Then one step is
    S[v]   = sum_{e->v} T[col_e]            (pure gather + segment-sum, no
                                             per-edge arithmetic at all)
    hX'[v] = dis_sh[v]*S[v] + xX[v]         (dis_sh = dis shifted by row.min())
    T'[v]  = dis[v]*hX'[v] = (dis*dis_sh)[v]*S[v] + dis[v]*xX[v]

Distribution: nodes dst-sharded over 8 cores (node v -> core v // (N/8)).
Each core owns a contiguous slice of a relabeled "slot" table; per-iteration
AllGather rebuilds the full table on every core. Gather of source rows uses
dma_gather (int16 indices -> table split in two halves; edges partitioned by
source half). Segment-sum runs on the TensorEngine: edges are binned into
sub-blocks of <=32 destination nodes with a fixed budget of 2 chunks (128
edges each) per source-half; each chunk's 0/1 selection matrix S (fp16,
host-built) is the stationary matmul operand, the gathered fp16 messages the
moving one, accumulating fp32 in PSUM.
"""

import numpy as np

# problem shape (hardcoded per the task contract)
N = 50000
E = 800000
IN_C = 128
HID = 128
OUT_C = 40
POWER1 = 8

NCORES = 8
SUB_NODES = 32          # destination slots per sub-block (= matmul M)
CHUNK = 128             # edges per chunk (= matmul K)
A_CHUNKS = 2            # chunks per sub-block from source half A
B_CHUNKS = 2
BATCH_GROUPS = 6        # psum groups (of 4 sub-blocks) per gather batch
GROUP_SUBS = 4          # sub-blocks per psum group ([128,128] psum tile)


# ----------------------------------------------------------------------------
# Host-side preprocessing
# ----------------------------------------------------------------------------

def _pack_core(degA, degB, capA, capB, sub_nodes):
    """2D best-fit-decreasing bin packing of nodes into sub-blocks."""
    order = np.argsort(-np.maximum(degA, degB), kind="stable")
    bins = []        # (node_list, sumA, sumB)
    for v in order:
        a, b = int(degA[v]), int(degB[v])
        best, best_slack = -1, None
        for i, (nodes, sa, sb) in enumerate(bins):
            if len(nodes) < sub_nodes and sa + a <= capA and sb + b <= capB:
                slack = (capA - sa - a) + (capB - sb - b)
                if best_slack is None or slack < best_slack:
                    best, best_slack = i, slack
        if best < 0:
            bins.append(([v], a, b))
        else:
            nodes, sa, sb = bins[best]
            nodes.append(v)
            bins[best] = (nodes, sa + a, sb + b)
    return [b[0] for b in bins]


def _preprocess(inputs, n=N, ncores=NCORES):
    x = np.asarray(inputs["x"], dtype=np.float32)
    edge_index = np.asarray(inputs["edge_index"])
    W_linX = np.asarray(inputs["W_linX"], dtype=np.float32)
    b_linX = np.asarray(inputs["b_linX"], dtype=np.float32)
    policy = np.asarray(inputs["policy"], dtype=np.float64)
    W_pred = np.asarray(inputs["W_pred"], dtype=np.float32)
    b_pred = np.asarray(inputs["b_pred"], dtype=np.float32)

    npc = n // ncores
    row = edge_index[0].astype(np.int64)
    col = edge_index[1].astype(np.int64)
    deg = np.bincount(col, minlength=n).astype(np.float64)
    dis = np.where(deg > 0, 1.0 / np.sqrt(np.maximum(deg, 1.0)), 0.0)
    shift = int(row.min())
    dst = row - shift                      # aggregation destination
    # per-dst factor is dis at the *unshifted* row id
    dis_sh = np.zeros(n, dtype=np.float64)
    hi = n - shift
    dis_sh[:hi] = dis[shift:]

    e = np.exp(policy[:2] - policy[:2].max())
    pp = e / e.sum()
    pp0, pp1 = float(pp[0]), float(pp[1])
    b_comb = pp0 + pp1                      # == 1.0, but don't rely on it

    half_nodes = (ncores // 2) * npc        # src half A = nodes < half_nodes
    capA, capB = A_CHUNKS * CHUNK, B_CHUNKS * CHUNK

    cores = []
    max_bins = 0
    for c in range(ncores):
        m = (dst >= c * npc) & (dst < (c + 1) * npc)
        e_dst = dst[m] - c * npc
        e_src = col[m]
        degA = np.bincount(e_dst[e_src < half_nodes], minlength=npc)
        degB = np.bincount(e_dst[e_src >= half_nodes], minlength=npc)
        bins = _pack_core(degA, degB, capA, capB, SUB_NODES)
        max_bins = max(max_bins, len(bins))
        cores.append((e_dst, e_src, bins))

    lcm = GROUP_SUBS * BATCH_GROUPS         # n_sub must divide into batches
    n_sub = -(-max_bins // lcm) * lcm
    slots = n_sub * SUB_NODES               # table rows per core
    tot = slots * ncores
    half_rows = tot // 2
    assert half_rows <= 32767, f"table half {half_rows} exceeds int16 range"
    n_grp = n_sub // GROUP_SUBS

    # slot assignment + global row map
    slot_of_node = np.full(n, -1, dtype=np.int64)
    for c, (e_dst, e_src, bins) in enumerate(cores):
        for bi, nodes in enumerate(bins):
            for k, v in enumerate(nodes):
                slot_of_node[c * npc + v] = c * slots + bi * SUB_NODES + k
    # an empty slot per core for padding gathers (guaranteed zero row)
    pad_row = np.zeros(ncores, dtype=np.int64)
    for c in range(ncores):
        used = np.zeros(slots, dtype=bool)
        sl = slot_of_node[c * npc:(c + 1) * npc] - c * slots
        used[sl[sl >= 0]] = True
        free = np.flatnonzero(~used)
        assert free.size > 0
        pad_row[c] = c * slots + free[0]

    per_core = []
    nchA, nchB = n_sub * A_CHUNKS, n_sub * B_CHUNKS
    for c, (e_dst, e_src, bins) in enumerate(cores):
        loc = slot_of_node[e_dst + c * npc] - c * slots
        dst_bin = loc // SUB_NODES
        dst_k = loc % SUB_NODES
        isA = e_src < half_nodes
        S = np.zeros((nchA + nchB, CHUNK, SUB_NODES), dtype=np.float16)
        idxA = np.full(nchA * CHUNK, pad_row[0], dtype=np.int64)
        idxB = np.full(nchB * CHUNK, pad_row[ncores // 2] - half_rows,
                       dtype=np.int64)
        for bi in range(n_sub):
            for half in (True, False):
                sel = (dst_bin == bi) & (isA == half)
                srcs = e_src[sel]
                dks = dst_k[sel]
                kk = len(srcs)
                cap = capA if half else capB
                assert kk <= cap, (c, bi, half, kk)
                if half:
                    cbase, sbase, idx, base = bi * A_CHUNKS, 0, idxA, 0
                else:
                    cbase, sbase, idx, base = bi * B_CHUNKS, nchA, idxB, half_rows
                for j in range(kk):
                    ch = cbase + j // CHUNK
                    S[sbase + ch, j % CHUNK, dks[j]] = 1.0
                rows = slot_of_node[srcs] - base
                idx[cbase * CHUNK: cbase * CHUNK + kk] = rows
        assert idxA.min() >= 0 and idxA.max() < half_rows
        assert idxB.min() >= 0 and idxB.max() < tot - half_rows

        def wrap_idx(idx):
            # index i consumed from [i % 16, i // 16]; replicate to 128 parts
            w = idx.reshape(-1, 16).T.astype(np.int16)      # [16, n/16]
            return np.tile(w, (8, 1))                        # [128, n/16]

        # slot-layout host arrays
        x_slot = np.zeros((slots, IN_C), dtype=np.float32)
        dis_slot = np.zeros(slots, dtype=np.float64)
        dsh_slot = np.zeros(slots, dtype=np.float64)
        nodes_c = np.arange(c * npc, (c + 1) * npc)
        sl = slot_of_node[nodes_c] - c * slots
        x_slot[sl] = x[nodes_c]
        dis_slot[sl] = dis[nodes_c]
        dsh_slot[sl] = dis_sh[nodes_c]

        grp = lambda v: v.reshape(n_grp, 128).T.astype(np.float32)
        per_core.append({
            "x_slot": x_slot,
            "S": np.ascontiguousarray(
                S.transpose(1, 0, 2).reshape(CHUNK, -1)),    # [128, TC*32]
            "idxA": wrap_idx(idxA),
            "idxB": wrap_idx(idxB),
            "disg": grp(dis_slot),
            "dis2g": grp(dis_slot * dsh_slot),
            "ag": grp((pp1 / b_comb) * dsh_slot),
            "W_linX": W_linX,
            "bX": np.tile(b_linX[None, :], (128, 1)).astype(np.float32),
            "W_pred": (b_comb * W_pred).astype(np.float32),
            "bP": np.tile(b_pred[None, :], (128, 1)).astype(np.float32),
            "ident": np.eye(128, dtype=np.float32),
        })

    meta = dict(n=n, ncores=ncores, npc=npc, n_sub=n_sub, n_grp=n_grp,
                slots=slots, tot=tot, half_rows=half_rows,
                slot_of_node=slot_of_node)
    return meta, per_core


# ----------------------------------------------------------------------------
# Bass program
# ----------------------------------------------------------------------------

def _build_program(meta, iters=POWER1):
    import concourse.bacc as bacc
    import concourse.mybir as mybir
    from concourse import tile

    f32, f16, i16 = mybir.dt.float32, mybir.dt.float16, mybir.dt.int16
    ADD, MULT = mybir.AluOpType.add, mybir.AluOpType.mult

    ncores = meta["ncores"]
    n_sub, n_grp = meta["n_sub"], meta["n_grp"]
    slots, tot, half_rows = meta["slots"], meta["tot"], meta["half_rows"]
    n_batches = n_grp // BATCH_GROUPS
    nchA = n_sub * A_CHUNKS
    batch_chunks = BATCH_GROUPS * GROUP_SUBS * A_CHUNKS      # 48
    batch_idx = batch_chunks * CHUNK                          # 6144
    TC = n_sub * (A_CHUNKS + B_CHUNKS)

    nc = bacc.Bacc("TRN2", target_bir_lowering=False, debug=False,
                   enable_asserts=False, num_devices=ncores,
                   num_swdge_queues=4)

    x_slot_h = nc.dram_tensor("x_slot", [slots, IN_C], f32, kind="ExternalInput")
    S_h = nc.dram_tensor("S", [CHUNK, TC * SUB_NODES], f16, kind="ExternalInput")
    idxA_h = nc.dram_tensor("idxA", [128, nchA * CHUNK // 16], i16,
                            kind="ExternalInput")
    idxB_h = nc.dram_tensor("idxB", [128, n_sub * B_CHUNKS * CHUNK // 16], i16,
                            kind="ExternalInput")
    disg_h = nc.dram_tensor("disg", [128, n_grp], f32, kind="ExternalInput")
    dis2g_h = nc.dram_tensor("dis2g", [128, n_grp], f32, kind="ExternalInput")
    ag_h = nc.dram_tensor("ag", [128, n_grp], f32, kind="ExternalInput")
    W_h = nc.dram_tensor("W_linX", [IN_C, HID], f32, kind="ExternalInput")
    bX_h = nc.dram_tensor("bX", [128, HID], f32, kind="ExternalInput")
    Wp_h = nc.dram_tensor("W_pred", [HID, OUT_C], f32, kind="ExternalInput")
    bP_h = nc.dram_tensor("bP", [128, OUT_C], f32, kind="ExternalInput")
    id_h = nc.dram_tensor("ident", [128, 128], f32, kind="ExternalInput")

    tabA = nc.dram_tensor("tabA", [tot, HID], f16, addr_space="Shared")
    tabB = nc.dram_tensor("tabB", [tot, HID], f16, addr_space="Shared")
    shard = nc.dram_tensor("shard", [slots, HID], f16)
    out_h = nc.dram_tensor("out", [slots, OUT_C], f32, kind="ExternalOutput")

    rg = [list(range(ncores))]

    with tile.TileContext(nc, num_cores=ncores) as tc:
        import contextlib
        with contextlib.ExitStack() as ctx:
            cpool = ctx.enter_context(tc.tile_pool(name="const", bufs=1))
            wpool = ctx.enter_context(tc.tile_pool(name="work", bufs=2))
            spool = ctx.enter_context(tc.tile_pool(name="stage", bufs=3))
            ppool = ctx.enter_context(
                tc.tile_pool(name="psum", bufs=4, space="PSUM"))
            tpool = ctx.enter_context(
                tc.tile_pool(name="psum2", bufs=2, space="PSUM"))

            # persistent SBUF
            S_sb = cpool.tile([CHUNK, TC * SUB_NODES], f16)
            nc.sync.dma_start(S_sb[:, :], S_h[:, :])
            idxA_sb = cpool.tile([128, nchA * CHUNK // 16], i16)
            nc.sync.dma_start(idxA_sb[:, :], idxA_h[:, :])
            idxB_sb = cpool.tile([128, n_sub * B_CHUNKS * CHUNK // 16], i16)
            nc.sync.dma_start(idxB_sb[:, :], idxB_h[:, :])
            disg = cpool.tile([128, n_grp], f32)
            nc.sync.dma_start(disg[:, :], disg_h[:, :])
            dis2g = cpool.tile([128, n_grp], f32)
            nc.sync.dma_start(dis2g[:, :], dis2g_h[:, :])
            ag = cpool.tile([128, n_grp], f32)
            nc.sync.dma_start(ag[:, :], ag_h[:, :])
            W_sb = cpool.tile([IN_C, HID], f32)
            nc.sync.dma_start(W_sb[:, :], W_h[:, :])
            bX_sb = cpool.tile([128, HID], f32)
            nc.sync.dma_start(bX_sb[:, :], bX_h[:, :])
            Wp_sb = cpool.tile([HID, OUT_C], f32)
            nc.sync.dma_start(Wp_sb[:, :], Wp_h[:, :])
            bP_sb = cpool.tile([128, OUT_C], f32)
            nc.sync.dma_start(bP_sb[:, :], bP_h[:, :])
            ident = cpool.tile([128, 128], f32)
            nc.sync.dma_start(ident[:, :], id_h[:, :])
            xX_sb = cpool.tile([128, n_grp * HID], f32)    # computed below

            # ---- prologue: xX = x @ W + b; T0 = dis * xX -> shard -> AG
            for g in range(n_grp):
                rows = slice(g * 128, (g + 1) * 128)
                gc = slice(g * HID, (g + 1) * HID)
                x_t = wpool.tile([128, IN_C], f32, tag="xt")
                nc.sync.dma_start(x_t[:, :], x_slot_h[rows, :])
                tp_ps = tpool.tile([128, 128], f32, tag="tp")
                nc.tensor.transpose(tp_ps[:, :], x_t[:, :], ident[:, :])
                xT_sb = wpool.tile([128, 128], f32, tag="xT")
                nc.vector.tensor_copy(xT_sb[:, :], tp_ps[:, :])
                mm_ps = tpool.tile([128, HID], f32, tag="mm2")
                nc.tensor.matmul(mm_ps[:, :], xT_sb[:, :], W_sb[:, :],
                                 start=True, stop=True)
                nc.vector.tensor_tensor(xX_sb[:, gc], mm_ps[:, :],
                                        bX_sb[:, :], op=ADD)
                stage = spool.tile([128, HID], f16, tag="stage")
                nc.vector.tensor_scalar_mul(stage[:, :], xX_sb[:, gc],
                                            disg[:, g:g + 1])
                nc.sync.dma_start(shard[rows, :], stage[:, :])

            nc.gpsimd.collective_compute(
                "AllGather", mybir.AluOpType.bypass, replica_groups=rg,
                ins=[shard.ap().opt()], outs=[tabA.ap().opt()])

            # ---- 8 SpMM iterations
            tables = [tabA, tabB]
            for t in range(iters):
                tin = tables[t % 2]
                last = t == iters - 1
                for b in range(n_batches):
                    mA = wpool.tile([128, batch_idx], f16, tag="mA")
                    nc.gpsimd.dma_gather(
                        mA[:, :].rearrange("p (c e) -> p c e", e=HID),
                        tin[0:half_rows, :],
                        idxA_sb[:, b * (batch_idx // 16):
                                (b + 1) * (batch_idx // 16)],
                        num_idxs=batch_idx, num_idxs_reg=batch_idx,
                        elem_size=HID, single_packet=False,
                        queue_num=(t * n_batches + b) % 2 * 2)
                    mB = wpool.tile([128, batch_idx], f16, tag="mB")
                    nc.gpsimd.dma_gather(
                        mB[:, :].rearrange("p (c e) -> p c e", e=HID),
                        tin[half_rows:tot, :],
                        idxB_sb[:, b * (batch_idx // 16):
                                (b + 1) * (batch_idx // 16)],
                        num_idxs=batch_idx, num_idxs_reg=batch_idx,
                        elem_size=HID, single_packet=False,
                        queue_num=(t * n_batches + b) % 2 * 2 + 1)
                    for u in range(BATCH_GROUPS):
                        g = b * BATCH_GROUPS + u
                        gc = slice(g * HID, (g + 1) * HID)
                        ps = ppool.tile([128, HID], f32, tag="ps")
                        for j in range(GROUP_SUBS):
                            sb = g * GROUP_SUBS + j
                            prange = slice(32 * j, 32 * j + 32)
                            tpos = (0, 32 * j)
                            for k in range(A_CHUNKS):
                                cA = sb * A_CHUNKS + k
                                q = (u * GROUP_SUBS + j) * A_CHUNKS + k
                                nc.tensor.matmul(
                                    ps[prange, :],
                                    S_sb[:, cA * 32:(cA + 1) * 32],
                                    mA[:, q * HID:(q + 1) * HID],
                                    start=(k == 0), stop=False,
                                    tile_position=tpos)
                            for k in range(B_CHUNKS):
                                cB = nchA + sb * B_CHUNKS + k
                                q = (u * GROUP_SUBS + j) * B_CHUNKS + k
                                nc.tensor.matmul(
                                    ps[prange, :],
                                    S_sb[:, cB * 32:(cB + 1) * 32],
                                    mB[:, q * HID:(q + 1) * HID],
                                    start=False, stop=(k == B_CHUNKS - 1),
                                    tile_position=tpos)
                        rows = slice(g * 128, (g + 1) * 128)
                        if not last:
                            t1 = wpool.tile([128, HID], f32, tag="t1")
                            nc.vector.tensor_scalar_mul(
                                t1[:, :], ps[:, :], dis2g[:, g:g + 1])
                            stage = spool.tile([128, HID], f16, tag="stage")
                            nc.vector.scalar_tensor_tensor(
                                stage[:, :], xX_sb[:, gc], disg[:, g:g + 1],
                                t1[:, :], op0=MULT, op1=ADD)
                            nc.sync.dma_start(shard[rows, :], stage[:, :])
                        else:
                            u_t = wpool.tile([128, HID], f32, tag="t1")
                            nc.vector.scalar_tensor_tensor(
                                u_t[:, :], ps[:, :], ag[:, g:g + 1],
                                xX_sb[:, gc], op0=MULT, op1=ADD)
                            nc.vector.tensor_scalar_max(u_t[:, :], u_t[:, :],
                                                        0.0)
                            tp_ps = tpool.tile([128, 128], f32, tag="tp")
                            nc.tensor.transpose(tp_ps[:, :], u_t[:, :],
                                                ident[:, :])
                            uT_sb = wpool.tile([128, 128], f32, tag="xT")
                            nc.vector.tensor_copy(uT_sb[:, :], tp_ps[:, :])
                            o_ps = tpool.tile([128, OUT_C], f32, tag="mm2")
                            nc.tensor.matmul(o_ps[:, :], uT_sb[:, :],
                                             Wp_sb[:, :], start=True, stop=True)
                            o_sb = spool.tile([128, OUT_C], f32, tag="osb")
                            nc.vector.tensor_tensor(o_sb[:, :], o_ps[:, :],
                                                    bP_sb[:, :], op=ADD)
                            nc.sync.dma_start(out_h[rows, :], o_sb[:, :])
                if not last:
                    nc.gpsimd.collective_compute(
                        "AllGather", mybir.AluOpType.bypass, replica_groups=rg,
                        ins=[shard.ap().opt()],
                        outs=[tables[(t + 1) % 2].ap().opt()])

    nc.compile()
    return nc


# ----------------------------------------------------------------------------
# Runner
# ----------------------------------------------------------------------------

def _run(inputs, n=N, ncores=NCORES, trace=False, use_sim=False, iters=POWER1):
    meta, per_core = _preprocess(inputs, n=n, ncores=ncores)
    nc = _build_program(meta, iters=iters)
    in_maps = [dict(pc) for pc in per_core]

    if use_sim:
        from concourse.bass_interp import MultiCoreSim
        sim = MultiCoreSim(nc, num_cores=ncores)
        for c in range(ncores):
            for k, v in in_maps[c].items():
                sim.cores[c].tensor(k)[:] = v
        sim.simulate(check_with_hw=False)
        results = [{"out": np.array(sim.cores[c].tensor("out"))}
                   for c in range(ncores)]
        bres = None
    else:
        from concourse.bass_utils import run_bass_kernel_spmd
        bres = run_bass_kernel_spmd(nc, in_maps, core_ids=list(range(ncores)),
                                    trace=trace)
        results = bres.results

    # unshard: slots -> nodes
    npc, slots = meta["npc"], meta["slots"]
    son = meta["slot_of_node"]
    out = np.zeros((n, OUT_C), dtype=np.float32)
    for c in range(ncores):
        nodes = np.arange(c * npc, (c + 1) * npc)
        out[nodes] = results[c]["out"][son[nodes] - c * slots]
    return out, bres


def kernel(**inputs) -> np.ndarray:
    # Run twice and compare: guards against rare transient device faults
    # (observed once after an unrecoverable-NRT event on a shared terminal).
    out1, _ = _run(inputs)
    out2, _ = _run(inputs)
    if np.allclose(out1, out2, rtol=0, atol=1e-4):
        return out1
    out3, _ = _run(inputs)
    if np.allclose(out1, out3, rtol=0, atol=1e-4):
        return out1
    return out2 if np.allclose(out2, out3, rtol=0, atol=1e-4) else out3



# revision 7
# speedup vs baseline: 2.0539x; 1.2838x over previous
"""GCN-style 8-step SpMM power iteration on 8 Trainium2 NeuronCores.

Math (reference):
    deg = segment_sum(1, col); dis = rsqrt(max(deg,1)) where deg>0 else 0
    norm_e = dis[row_e] * dis[col_e];  row' = row - row.min()
    xX = x @ W_linX + b_linX
    hX_{t+1}[v] = sum_{e: row'_e = v} norm_e * hX_t[col_e] + xX[v]   (8 times)
    out = relu(pp0*xX + pp1*hX_8) @ W_pred + b_pred

Key algebraic trick: norm factorizes per-edge into src/dst node factors, so we
keep the node table pre-scaled: T = dis ⊙ hX. Then one step is
    S[v]   = sum_{e->v} T[col_e]            (pure gather + segment-sum, no
                                             per-edge arithmetic at all)
    hX'[v] = dis_sh[v]*S[v] + xX[v]         (dis_sh = dis shifted by row.min())
    T'[v]  = dis[v]*hX'[v] = (dis*dis_sh)[v]*S[v] + dis[v]*xX[v]

Distribution: nodes dst-sharded over 8 cores (node v -> core v // (N/8)).
Each core owns a contiguous slice of a relabeled "slot" table; per-iteration
AllGather rebuilds the full table on every core. Gather of source rows uses
dma_gather (int16 indices -> table split in two halves; edges partitioned by
source half). Segment-sum runs on the TensorEngine: edges are binned into
sub-blocks of <=32 destination nodes with a fixed budget of 2 chunks (128
edges each) per source-half; each chunk's 0/1 selection matrix S (fp16,
host-built) is the stationary matmul operand, the gathered fp16 messages the
moving one, accumulating fp32 in PSUM.
"""

import numpy as np

# problem shape (hardcoded per the task contract)
N = 50000
E = 800000
IN_C = 128
HID = 128
OUT_C = 40
POWER1 = 8

NCORES = 8
SUB_NODES = 32          # destination slots per sub-block (= matmul M)
CHUNK = 128             # edges per chunk (= matmul K)
A_CHUNKS = 2            # chunks per sub-block from source half A
B_CHUNKS = 2
BATCH_GROUPS = 6        # psum groups (of 4 sub-blocks) per gather batch
GROUP_SUBS = 4          # sub-blocks per psum group ([128,128] psum tile)


# ----------------------------------------------------------------------------
# Host-side preprocessing
# ----------------------------------------------------------------------------

def _pack_core(degA, degB, capA, capB, sub_nodes):
    """2D best-fit-decreasing bin packing of nodes into sub-blocks."""
    order = np.argsort(-np.maximum(degA, degB), kind="stable")
    bins = []        # (node_list, sumA, sumB)
    for v in order:
        a, b = int(degA[v]), int(degB[v])
        best, best_slack = -1, None
        for i, (nodes, sa, sb) in enumerate(bins):
            if len(nodes) < sub_nodes and sa + a <= capA and sb + b <= capB:
                slack = (capA - sa - a) + (capB - sb - b)
                if best_slack is None or slack < best_slack:
                    best, best_slack = i, slack
        if best < 0:
            bins.append(([v], a, b))
        else:
            nodes, sa, sb = bins[best]
            nodes.append(v)
            bins[best] = (nodes, sa + a, sb + b)
    return [b[0] for b in bins]


def _preprocess(inputs, n=N, ncores=NCORES):
    x = np.asarray(inputs["x"], dtype=np.float32)
    edge_index = np.asarray(inputs["edge_index"])
    W_linX = np.asarray(inputs["W_linX"], dtype=np.float32)
    b_linX = np.asarray(inputs["b_linX"], dtype=np.float32)
    policy = np.asarray(inputs["policy"], dtype=np.float64)
    W_pred = np.asarray(inputs["W_pred"], dtype=np.float32)
    b_pred = np.asarray(inputs["b_pred"], dtype=np.float32)

    npc = n // ncores
    row = edge_index[0].astype(np.int64)
    col = edge_index[1].astype(np.int64)
    deg = np.bincount(col, minlength=n).astype(np.float64)
    dis = np.where(deg > 0, 1.0 / np.sqrt(np.maximum(deg, 1.0)), 0.0)
    shift = int(row.min())
    dst = row - shift                      # aggregation destination
    # per-dst factor is dis at the *unshifted* row id
    dis_sh = np.zeros(n, dtype=np.float64)
    hi = n - shift
    dis_sh[:hi] = dis[shift:]

    e = np.exp(policy[:2] - policy[:2].max())
    pp = e / e.sum()
    pp0, pp1 = float(pp[0]), float(pp[1])
    b_comb = pp0 + pp1                      # == 1.0, but don't rely on it
    assert pp1 > 0

    half_nodes = (ncores // 2) * npc        # src half A = nodes < half_nodes
    capA, capB = A_CHUNKS * CHUNK, B_CHUNKS * CHUNK

    cores = []
    max_bins = 0
    for c in range(ncores):
        m = (dst >= c * npc) & (dst < (c + 1) * npc)
        e_dst = dst[m] - c * npc
        e_src = col[m]
        degA = np.bincount(e_dst[e_src < half_nodes], minlength=npc)
        degB = np.bincount(e_dst[e_src >= half_nodes], minlength=npc)
        bins = _pack_core(degA, degB, capA, capB, SUB_NODES)
        max_bins = max(max_bins, len(bins))
        cores.append((e_dst, e_src, bins))

    lcm = GROUP_SUBS * BATCH_GROUPS         # n_sub must divide into batches
    n_sub = -(-max_bins // lcm) * lcm
    slots = n_sub * SUB_NODES               # table rows per core
    tot = slots * ncores
    half_rows = tot // 2
    assert half_rows <= 32767, f"table half {half_rows} exceeds int16 range"
    n_grp = n_sub // GROUP_SUBS

    # slot assignment + global row map
    slot_of_node = np.full(n, -1, dtype=np.int64)
    for c, (e_dst, e_src, bins) in enumerate(cores):
        for bi, nodes in enumerate(bins):
            for k, v in enumerate(nodes):
                slot_of_node[c * npc + v] = c * slots + bi * SUB_NODES + k
    # an empty slot per core for padding gathers (guaranteed zero row)
    pad_row = np.zeros(ncores, dtype=np.int64)
    for c in range(ncores):
        used = np.zeros(slots, dtype=bool)
        sl = slot_of_node[c * npc:(c + 1) * npc] - c * slots
        used[sl[sl >= 0]] = True
        free = np.flatnonzero(~used)
        assert free.size > 0
        pad_row[c] = c * slots + free[0]

    per_core = []
    nchA, nchB = n_sub * A_CHUNKS, n_sub * B_CHUNKS
    for c, (e_dst, e_src, bins) in enumerate(cores):
        loc = slot_of_node[e_dst + c * npc] - c * slots
        dst_bin = loc // SUB_NODES
        dst_k = loc % SUB_NODES
        isA = e_src < half_nodes
        S = np.zeros((nchA + nchB, CHUNK, SUB_NODES), dtype=np.float16)
        idxA = np.full(nchA * CHUNK, pad_row[0], dtype=np.int64)
        idxB = np.full(nchB * CHUNK, pad_row[ncores // 2] - half_rows,
                       dtype=np.int64)
        for bi in range(n_sub):
            for half in (True, False):
                sel = (dst_bin == bi) & (isA == half)
                srcs = e_src[sel]
                dks = dst_k[sel]
                dstv = e_dst[sel] + c * npc          # global dst node ids
                kk = len(srcs)
                cap = capA if half else capB
                assert kk <= cap, (c, bi, half, kk)
                if half:
                    cbase, sbase, idx, base = bi * A_CHUNKS, 0, idxA, 0
                else:
                    cbase, sbase, idx, base = bi * B_CHUNKS, nchA, idxB, half_rows
                jj = np.arange(kk)
                S[sbase + cbase + jj // CHUNK, jj % CHUNK, dks] = \
                    dis_sh[dstv].astype(np.float16)
                rows = slot_of_node[srcs] - base
                idx[cbase * CHUNK: cbase * CHUNK + kk] = rows
        assert idxA.min() >= 0 and idxA.max() < half_rows
        assert idxB.min() >= 0 and idxB.max() < tot - half_rows

        def wrap_idx(idx):
            # index i consumed from [i % 16, i // 16]; replicate to 128 parts
            w = idx.reshape(-1, 16).T.astype(np.int16)      # [16, n/16]
            return np.tile(w, (8, 1))                        # [128, n/16]

        # slot-layout host arrays
        x_slot = np.zeros((slots, IN_C), dtype=np.float32)
        dis_slot = np.zeros(slots, dtype=np.float64)
        dsh_slot = np.zeros(slots, dtype=np.float64)
        nodes_c = np.arange(c * npc, (c + 1) * npc)
        sl = slot_of_node[nodes_c] - c * slots
        x_slot[sl] = x[nodes_c]
        dis_slot[sl] = dis[nodes_c]
        dsh_slot[sl] = dis_sh[nodes_c]

        grp = lambda v: v.reshape(n_grp, 128).T.astype(np.float32)
        per_core.append({
            "x_slot": x_slot,
            "S": np.ascontiguousarray(
                S.transpose(1, 0, 2).reshape(CHUNK, -1)),    # [128, TC*32]
            "idxA": wrap_idx(idxA),
            "idxB": wrap_idx(idxB),
            "disg": grp(dis_slot),
            "W_linX": W_linX,
            "bX": np.tile(b_linX[None, :], (128, 1)).astype(np.float32),
            "W_pred": (b_comb * W_pred).astype(np.float32),
            "bP": np.tile(b_pred[None, :], (128, 1)).astype(np.float32),
            "ident": np.eye(128, dtype=np.float32),
            "ident16": np.eye(128, dtype=np.float16),
        })

    meta = dict(n=n, ncores=ncores, npc=npc, n_sub=n_sub, n_grp=n_grp,
                slots=slots, tot=tot, half_rows=half_rows,
                slot_of_node=slot_of_node, pp1_over_b=pp1 / b_comb,
                b_over_pp1=b_comb / pp1)
    return meta, per_core


# ----------------------------------------------------------------------------
# Bass program
# ----------------------------------------------------------------------------

def _build_program(meta, iters=POWER1):
    import concourse.bacc as bacc
    import concourse.mybir as mybir
    from concourse import tile

    f32, f16, i16 = mybir.dt.float32, mybir.dt.float16, mybir.dt.int16
    ADD, MULT = mybir.AluOpType.add, mybir.AluOpType.mult
    Copy = mybir.ActivationFunctionType.Copy
    Relu = mybir.ActivationFunctionType.Relu

    ncores = meta["ncores"]
    n_sub, n_grp = meta["n_sub"], meta["n_grp"]
    slots, tot, half_rows = meta["slots"], meta["tot"], meta["half_rows"]
    pp1_over_b = meta["pp1_over_b"]
    b_over_pp1 = meta["b_over_pp1"]
    n_batches = n_grp // BATCH_GROUPS
    nchA = n_sub * A_CHUNKS
    batch_chunks = BATCH_GROUPS * GROUP_SUBS * A_CHUNKS      # 48
    batch_idx = batch_chunks * CHUNK                          # 6144
    TC = n_sub * (A_CHUNKS + B_CHUNKS)

    nc = bacc.Bacc("TRN2", target_bir_lowering=False, debug=False,
                   enable_asserts=False, num_devices=ncores,
                   num_swdge_queues=4)

    x_slot_h = nc.dram_tensor("x_slot", [slots, IN_C], f32, kind="ExternalInput")
    S_h = nc.dram_tensor("S", [CHUNK, TC * SUB_NODES], f16, kind="ExternalInput")
    idxA_h = nc.dram_tensor("idxA", [128, nchA * CHUNK // 16], i16,
                            kind="ExternalInput")
    idxB_h = nc.dram_tensor("idxB", [128, n_sub * B_CHUNKS * CHUNK // 16], i16,
                            kind="ExternalInput")
    disg_h = nc.dram_tensor("disg", [128, n_grp], f32, kind="ExternalInput")
    W_h = nc.dram_tensor("W_linX", [IN_C, HID], f32, kind="ExternalInput")
    bX_h = nc.dram_tensor("bX", [128, HID], f32, kind="ExternalInput")
    Wp_h = nc.dram_tensor("W_pred", [HID, OUT_C], f32, kind="ExternalInput")
    bP_h = nc.dram_tensor("bP", [128, OUT_C], f32, kind="ExternalInput")
    id_h = nc.dram_tensor("ident", [128, 128], f32, kind="ExternalInput")
    id16_h = nc.dram_tensor("ident16", [128, 128], f16, kind="ExternalInput")

    tabA = nc.dram_tensor("tabA", [tot, HID], f16, addr_space="Shared")
    tabB = nc.dram_tensor("tabB", [tot, HID], f16, addr_space="Shared")
    shard = nc.dram_tensor("shard", [slots, HID], f16)
    out_h = nc.dram_tensor("out", [slots, OUT_C], f32, kind="ExternalOutput")

    rg = [list(range(ncores))]

    with tile.TileContext(nc, num_cores=ncores) as tc:
        import contextlib
        with contextlib.ExitStack() as ctx:
            cpool = ctx.enter_context(tc.tile_pool(name="const", bufs=1))
            wpool = ctx.enter_context(tc.tile_pool(name="work", bufs=2))
            spool = ctx.enter_context(tc.tile_pool(name="stage", bufs=3))
            gpool = ctx.enter_context(tc.tile_pool(name="gath", bufs=3))
            ppool = ctx.enter_context(
                tc.tile_pool(name="psum", bufs=4, space="PSUM"))
            tpool = ctx.enter_context(
                tc.tile_pool(name="psum2", bufs=2, space="PSUM"))

            # persistent SBUF
            S_sb = cpool.tile([CHUNK, TC * SUB_NODES], f16)
            nc.sync.dma_start(S_sb[:, :], S_h[:, :])
            idxA_sb = cpool.tile([128, nchA * CHUNK // 16], i16)
            nc.sync.dma_start(idxA_sb[:, :], idxA_h[:, :])
            idxB_sb = cpool.tile([128, n_sub * B_CHUNKS * CHUNK // 16], i16)
            nc.sync.dma_start(idxB_sb[:, :], idxB_h[:, :])
            disg = cpool.tile([128, n_grp], f32)
            nc.sync.dma_start(disg[:, :], disg_h[:, :])
            W_sb = cpool.tile([IN_C, HID], f32)
            nc.sync.dma_start(W_sb[:, :], W_h[:, :])
            bX_sb = cpool.tile([128, HID], f32)
            nc.sync.dma_start(bX_sb[:, :], bX_h[:, :])
            Wp_sb = cpool.tile([HID, OUT_C], f32)
            nc.sync.dma_start(Wp_sb[:, :], Wp_h[:, :])
            bP_sb = cpool.tile([128, OUT_C], f32)
            nc.sync.dma_start(bP_sb[:, :], bP_h[:, :])
            ident = cpool.tile([128, 128], f32)
            nc.sync.dma_start(ident[:, :], id_h[:, :])
            id16 = cpool.tile([128, 128], f16)
            nc.sync.dma_start(id16[:, :], id16_h[:, :])
            # fp16 copies of xX: plain (iters 0..6) and scaled by b/pp1 (last)
            xX16 = cpool.tile([128, n_grp * HID], f16)
            xXb16 = cpool.tile([128, n_grp * HID], f16)

            # ---- prologue: xX = x @ W + b; T0 = dis * xX -> shard -> AG
            for g in range(n_grp):
                rows = slice(g * 128, (g + 1) * 128)
                gc = slice(g * HID, (g + 1) * HID)
                x_t = wpool.tile([128, IN_C], f32, tag="xt")
                nc.sync.dma_start(x_t[:, :], x_slot_h[rows, :])
                tp_ps = tpool.tile([128, 128], f32, tag="tp")
                nc.tensor.transpose(tp_ps[:, :], x_t[:, :], ident[:, :])
                xT_sb = wpool.tile([128, 128], f32, tag="xT")
                nc.vector.tensor_copy(xT_sb[:, :], tp_ps[:, :])
                mm_ps = tpool.tile([128, HID], f32, tag="mm2")
                nc.tensor.matmul(mm_ps[:, :], xT_sb[:, :], W_sb[:, :],
                                 start=True, stop=True)
                xX32 = wpool.tile([128, HID], f32, tag="t1")
                nc.vector.tensor_tensor(xX32[:, :], mm_ps[:, :],
                                        bX_sb[:, :], op=ADD)
                nc.vector.tensor_scalar_mul(xX16[:, gc], xX32[:, :], 1.0)
                nc.vector.tensor_scalar_mul(xXb16[:, gc], xX32[:, :],
                                            float(b_over_pp1))
                stage = spool.tile([128, HID], f16, tag="stage")
                nc.vector.tensor_scalar_mul(stage[:, :], xX32[:, :],
                                            disg[:, g:g + 1])
                nc.sync.dma_start(shard[rows, :], stage[:, :])

            nc.gpsimd.collective_compute(
                "AllGather", mybir.AluOpType.bypass, replica_groups=rg,
                ins=[shard.ap().opt()], outs=[tabA.ap().opt()])

            # ---- 8 SpMM iterations
            tables = [tabA, tabB]
            for t in range(iters):
                tin = tables[t % 2]
                last = t == iters - 1
                for b in range(n_batches):
                    mA = gpool.tile([128, batch_idx], f16, tag="mA")
                    nc.gpsimd.dma_gather(
                        mA[:, :].rearrange("p (c e) -> p c e", e=HID),
                        tin[0:half_rows, :],
                        idxA_sb[:, b * (batch_idx // 16):
                                (b + 1) * (batch_idx // 16)],
                        num_idxs=batch_idx, num_idxs_reg=batch_idx,
                        elem_size=HID, single_packet=False,
                        queue_num=(t * n_batches + b) % 2 * 2)
                    mB = gpool.tile([128, batch_idx], f16, tag="mB")
                    nc.gpsimd.dma_gather(
                        mB[:, :].rearrange("p (c e) -> p c e", e=HID),
                        tin[half_rows:tot, :],
                        idxB_sb[:, b * (batch_idx // 16):
                                (b + 1) * (batch_idx // 16)],
                        num_idxs=batch_idx, num_idxs_reg=batch_idx,
                        elem_size=HID, single_packet=False,
                        queue_num=(t * n_batches + b) % 2 * 2 + 1)
                    for u in range(BATCH_GROUPS):
                        g = b * BATCH_GROUPS + u
                        gc = slice(g * HID, (g + 1) * HID)
                        ps = ppool.tile([128, HID], f32, tag="ps")
                        # seed psum with xX (or (b/pp1)*xX on the last pass)
                        nc.tensor.matmul(ps[:, :], id16[:, :],
                                         (xXb16 if last else xX16)[:, gc],
                                         start=True, stop=False)
                        for j in range(GROUP_SUBS):
                            sb = g * GROUP_SUBS + j
                            prange = slice(32 * j, 32 * j + 32)
                            tpos = (0, 32 * j)
                            for k in range(A_CHUNKS):
                                cA = sb * A_CHUNKS + k
                                q = (u * GROUP_SUBS + j) * A_CHUNKS + k
                                nc.tensor.matmul(
                                    ps[prange, :],
                                    S_sb[:, cA * 32:(cA + 1) * 32],
                                    mA[:, q * HID:(q + 1) * HID],
                                    start=False, stop=False,
                                    tile_position=tpos)
                            for k in range(B_CHUNKS):
                                cB = nchA + sb * B_CHUNKS + k
                                q = (u * GROUP_SUBS + j) * B_CHUNKS + k
                                nc.tensor.matmul(
                                    ps[prange, :],
                                    S_sb[:, cB * 32:(cB + 1) * 32],
                                    mB[:, q * HID:(q + 1) * HID],
                                    start=False, stop=(k == B_CHUNKS - 1),
                                    tile_position=tpos)
                        rows = slice(g * 128, (g + 1) * 128)
                        if not last:
                            # psum = dis_sh*sum + xX = hX'; stage = dis*psum
                            stage = spool.tile([128, HID], f16, tag="stage")
                            nc.scalar.activation(stage[:, :], ps[:, :], Copy,
                                                 bias=0.0,
                                                 scale=disg[:, g:g + 1])
                            nc.sync.dma_start(shard[rows, :], stage[:, :])
                        else:
                            # psum = dis_sh*sum + (b/pp1)*xX
                            # u = relu((pp1/b)*psum) = relu(h)/b
                            u_t = wpool.tile([128, HID], f32, tag="t1")
                            nc.scalar.activation(u_t[:, :], ps[:, :], Relu,
                                                 bias=0.0,
                                                 scale=float(pp1_over_b))
                            tp_ps = tpool.tile([128, 128], f32, tag="tp")
                            nc.tensor.transpose(tp_ps[:, :], u_t[:, :],
                                                ident[:, :])
                            uT_sb = wpool.tile([128, 128], f32, tag="xT")
                            nc.vector.tensor_copy(uT_sb[:, :], tp_ps[:, :])
                            o_ps = tpool.tile([128, OUT_C], f32, tag="mm2")
                            nc.tensor.matmul(o_ps[:, :], uT_sb[:, :],
                                             Wp_sb[:, :], start=True, stop=True)
                            o_sb = spool.tile([128, OUT_C], f32, tag="osb")
                            nc.vector.tensor_tensor(o_sb[:, :], o_ps[:, :],
                                                    bP_sb[:, :], op=ADD)
                            nc.sync.dma_start(out_h[rows, :], o_sb[:, :])
                if not last:
                    nc.gpsimd.collective_compute(
                        "AllGather", mybir.AluOpType.bypass, replica_groups=rg,
                        ins=[shard.ap().opt()],
                        outs=[tables[(t + 1) % 2].ap().opt()])

    nc.compile()
    return nc


# ----------------------------------------------------------------------------
# Runner
# ----------------------------------------------------------------------------

def _run(inputs, n=N, ncores=NCORES, trace=False, use_sim=False, iters=POWER1):
    meta, per_core = _preprocess(inputs, n=n, ncores=ncores)
    nc = _build_program(meta, iters=iters)
    in_maps = [dict(pc) for pc in per_core]

    if use_sim:
        from concourse.bass_interp import MultiCoreSim
        sim = MultiCoreSim(nc, num_cores=ncores)
        for c in range(ncores):
            for k, v in in_maps[c].items():
                sim.cores[c].tensor(k)[:] = v
        sim.simulate(check_with_hw=False)
        results = [{"out": np.array(sim.cores[c].tensor("out"))}
                   for c in range(ncores)]
        bres = None
    else:
        from concourse.bass_utils import run_bass_kernel_spmd
        bres = run_bass_kernel_spmd(nc, in_maps, core_ids=list(range(ncores)),
                                    trace=trace)
        results = bres.results

    # unshard: slots -> nodes
    npc, slots = meta["npc"], meta["slots"]
    son = meta["slot_of_node"]
    out = np.zeros((n, OUT_C), dtype=np.float32)
    for c in range(ncores):
        nodes = np.arange(c * npc, (c + 1) * npc)
        out[nodes] = results[c]["out"][son[nodes] - c * slots]
    return out, bres


def kernel(**inputs) -> np.ndarray:
    # Run twice and compare: guards against rare transient device faults
    # (observed once after an unrecoverable-NRT event on a shared terminal).
    out1, _ = _run(inputs)
    out2, _ = _run(inputs)
    if np.allclose(out1, out2, rtol=0, atol=1e-4):
        return out1
    out3, _ = _run(inputs)
    if np.allclose(out1, out3, rtol=0, atol=1e-4):
        return out1
    return out2 if np.allclose(out2, out3, rtol=0, atol=1e-4) else out3

